# revision 3
# baseline (speedup 1.0000x reference)
"""Trainium2 Bass kernel for CounterfactualMultiheadAttention (uniform type).

Math note: the reference computes w = softmax(masked scores), then replaces
every nonzero weight with row_sum(w)/nnz(w).  Softmax rows sum to 1 and are
nonzero exactly at unmasked key positions (score spread << 87, so exp never
underflows to 0), hence

    attn_weights[b,h,q,k] = (1 - mask[b,k]) / cnt[b]        (indep. of h, q)
    ctx[b,h,q,:]          = masked-mean over k of V[b,k,:]   (indep. of q)
    output[b,q,:] = (masked_mean(value[b]) @ Wv.T + bv) @ Wo.T + bo

so query/key/Wq/bq/Wk/bk cancel out entirely.  The kernel computes the
masked mean + two matvecs on-device and materializes the broadcast outputs.

Sharding over 8 cores: core c handles batch b=c//2; writes attn_weights for
heads [8*(c%2), 8*(c%2)+8) of that batch and query-rows [512*(c%2), ...+512)
of output[b].
"""

import sys

for _p in ("/opt/trn_rl_repo",):
    if _p not in sys.path:
        sys.path.insert(0, _p)

from contextlib import ExitStack

import numpy as np

import concourse.bass as bass
import concourse.mybir as mybir
import concourse.tile as tile
from concourse import bacc
from concourse.bass_utils import run_bass_kernel_spmd
from concourse.masks import make_identity

B, L, D = 4, 1024, 1024
H, HPC = 16, 8  # total heads, heads per core
P = 128
NCH = D // P  # 8 chunks of 128
F32 = mybir.dt.float32
U8 = mybir.dt.uint8
ALU = mybir.AluOpType


def build_program():
    nc = bacc.Bacc("TRN2", target_bir_lowering=False, debug=False, num_devices=8)

    value_b = nc.dram_tensor("value_b", [L, D], F32, kind="ExternalInput")
    mask_b = nc.dram_tensor("mask_b", [L], U8, kind="ExternalInput")
    wv = nc.dram_tensor("wv", [D, D], F32, kind="ExternalInput")
    bv = nc.dram_tensor("bv", [D], F32, kind="ExternalInput")
    wo = nc.dram_tensor("wo", [D, D], F32, kind="ExternalInput")
    bo = nc.dram_tensor("bo", [D], F32, kind="ExternalInput")
    attn_out = nc.dram_tensor("attn_out", [HPC, L, L], F32, kind="ExternalOutput")
    out_part = nc.dram_tensor("out_part", [L // 2, D], F32, kind="ExternalOutput")

    with tile.TileContext(nc) as tc, ExitStack() as ctx:
        singles = ctx.enter_context(tc.tile_pool(name="singles", bufs=1))
        psum = ctx.enter_context(tc.tile_pool(name="psum", bufs=1, space="PSUM"))

        # ---- attn row = (1 - mask) / cnt, replicated on all 128 partitions ----
        mask_row = singles.tile([P, L], U8)
        mask_bcast_ap = bass.AP(tensor=mask_b, offset=0, ap=[[0, P], [1, L]])
        nc.scalar.dma_start(out=mask_row[:], in_=mask_bcast_ap)
        valid_row = singles.tile([P, L], F32)
        nc.vector.tensor_scalar(
            out=valid_row[:], in0=mask_row[:], scalar1=0, scalar2=None,
            op0=ALU.is_equal,
        )
        cnt = singles.tile([P, 1], F32)
        nc.vector.reduce_sum(out=cnt[:], in_=valid_row[:], axis=mybir.AxisListType.X)
        inv = singles.tile([P, 1], F32)
        nc.vector.reciprocal(out=inv[:], in_=cnt[:])
        # 8 replicas of the row along the free dim -> one 4 MiB source tile
        big = singles.tile([P, NCH, L], F32)
        for n in range(NCH):
            nc.vector.tensor_scalar_mul(big[:, n, :], valid_row[:], inv[:])

        # ---- attn_weights: 8 head-slabs x 4 MiB broadcast-writes ----
        attn_view = attn_out.ap().rearrange("h (n p) d -> h p n d", p=P)
        for h in range(HPC):
            nc.sync.dma_start(out=attn_view[h], in_=big[:])

        # ---- masked mean of value rows (PE contraction over k) ----
        wcol_u8 = singles.tile([P, NCH], U8)
        nc.scalar.dma_start(out=wcol_u8[:], in_=mask_b.ap().rearrange("(n p) -> p n", p=P))
        wcol = singles.tile([P, NCH], F32)
        nc.vector.tensor_scalar(
            out=wcol[:], in0=wcol_u8[:], scalar1=0, scalar2=None, op0=ALU.is_equal,
        )
        wcoln = singles.tile([P, NCH], F32)
        nc.vector.tensor_scalar_mul(wcoln[:], wcol[:], inv[:])

        value_sb = singles.tile([P, NCH, D], F32)
        nc.scalar.dma_start(out=value_sb[:], in_=value_b.ap().rearrange("(n p) d -> p n d", p=P))
        mv_psum = psum.tile([1, L], F32)
        for half in range(2):
            sl = slice(half * 512, (half + 1) * 512)
            for n in range(NCH):
                nc.tensor.matmul(
                    mv_psum[:, sl], lhsT=wcoln[:, n : n + 1], rhs=value_sb[:, n, sl],
                    start=(n == 0), stop=(n == NCH - 1),
                )
        mv_row = singles.tile([1, L], F32)
        nc.scalar.copy(out=mv_row[:], in_=mv_psum[:].rearrange("o f -> o f"))
        mv_bcast = singles.tile([P, L], F32)
        nc.gpsimd.partition_broadcast(mv_bcast[:], mv_row[:])

        # ---- vbar = mv @ Wv.T + bv  (DVE dot-rows, column-layout result) ----
        wv_tile = singles.tile([P, NCH, D], F32)
        nc.scalar.dma_start(out=wv_tile[:], in_=wv.ap().rearrange("(n p) d -> p n d", p=P))
        bv_col = singles.tile([P, NCH], F32)
        nc.scalar.dma_start(out=bv_col[:], in_=bv.ap().rearrange("(n p) -> p n", p=P))
        vbar_acc = singles.tile([P, NCH], F32)
        scratch = singles.tile([P, D], F32)
        for n in range(NCH):
            nc.vector.tensor_mul(scratch[:], wv_tile[:, n, :], mv_bcast[:])
            nc.vector.reduce_sum(out=vbar_acc[:, n : n + 1], in_=scratch[:],
                                 axis=mybir.AxisListType.X)
        vbar_col = singles.tile([P, NCH], F32)
        nc.vector.tensor_add(vbar_col[:], vbar_acc[:], bv_col[:])

        # column [128, 8] -> row [1, 1024] via PE transpose, then broadcast
        identity = singles.tile([P, P], F32)
        make_identity(nc, identity[:])
        vbar_rpsum = psum.tile([1, NCH, P], F32)
        for n in range(NCH):
            nc.tensor.transpose(vbar_rpsum[:, n, :], vbar_col[:, n : n + 1], identity[:])
        vbar_row = singles.tile([1, D], F32)
        nc.scalar.copy(out=vbar_row[:], in_=vbar_rpsum[:].rearrange("o n p -> o (n p)"))
        vbar_bcast = singles.tile([P, D], F32)
        nc.gpsimd.partition_broadcast(vbar_bcast[:], vbar_row[:])

        # ---- orow = vbar @ Wo.T + bo ----
        wo_tile = singles.tile([P, NCH, D], F32)
        nc.scalar.dma_start(out=wo_tile[:], in_=wo.ap().rearrange("(m p) j -> p m j", p=P))
        bo_col = singles.tile([P, NCH], F32)
        nc.scalar.dma_start(out=bo_col[:], in_=bo.ap().rearrange("(m p) -> p m", p=P))
        orow_acc = singles.tile([P, NCH], F32)
        scratch2 = singles.tile([P, D], F32)
        for m in range(NCH):
            nc.vector.tensor_mul(scratch2[:], wo_tile[:, m, :], vbar_bcast[:])
            nc.vector.reduce_sum(out=orow_acc[:, m : m + 1], in_=scratch2[:],
                                 axis=mybir.AxisListType.X)
        orow_col = singles.tile([P, NCH], F32)
        nc.vector.tensor_add(orow_col[:], orow_acc[:], bo_col[:])
        orow_rpsum = psum.tile([1, NCH, P], F32)
        for m in range(NCH):
            nc.tensor.transpose(orow_rpsum[:, m, :], orow_col[:, m : m + 1], identity[:])
        orow_row = singles.tile([1, D], F32)
        nc.scalar.copy(out=orow_row[:], in_=orow_rpsum[:].rearrange("o n p -> o (n p)"))
        orow_bcast = singles.tile([P, D], F32)
        nc.gpsimd.partition_broadcast(orow_bcast[:], orow_row[:])

        # ---- output rows: 4 row-block broadcast-writes (512 q rows) ----
        out_view = out_part.ap().rearrange("(n p) d -> n p d", p=P)
        for n in range(L // 2 // P):
            nc.sync.dma_start(out=out_view[n], in_=orow_bcast[:])

    nc.compile()
    return nc


_NC = None


def _get_nc():
    global _NC
    if _NC is None:
        _NC = build_program()
    return _NC


_LAST_RESULTS = None  # BassKernelResults of the most recent run (for profiling)


def kernel(query, key, value, key_padding_mask, Wq, bq, Wk, bk, Wv, bv, Wo, bo,
           _trace=False):
    nc = _get_nc()
    Wv = np.ascontiguousarray(np.asarray(Wv, dtype=np.float32))
    bvv = np.ascontiguousarray(np.asarray(bv, dtype=np.float32))
    Wo = np.ascontiguousarray(np.asarray(Wo, dtype=np.float32))
    boo = np.ascontiguousarray(np.asarray(bo, dtype=np.float32))
    value = np.asarray(value, dtype=np.float32)
    masku8 = np.asarray(key_padding_mask).astype(np.uint8)

    in_maps = []
    for c in range(8):
        b = c // 2
        in_maps.append({
            "value_b": np.ascontiguousarray(value[b]),
            "mask_b": np.ascontiguousarray(masku8[b]),
            "wv": Wv, "bv": bvv, "wo": Wo, "bo": boo,
        })

    global _LAST_RESULTS
    _LAST_RESULTS = run_bass_kernel_spmd(
        nc, in_maps, core_ids=list(range(8)), trace=_trace,
    )
    res = _LAST_RESULTS.results

    output = np.empty((B, L, D), dtype=np.float32)
    attn = np.empty((B, H, L, L), dtype=np.float32)
    for c in range(8):
        b, hg = c // 2, c % 2
        attn[b, hg * HPC : (hg + 1) * HPC] = res[c]["attn_out"]
        output[b, hg * 512 : (hg + 1) * 512] = res[c]["out_part"]
    return output, attn


# revision 10
# speedup vs baseline: 1.3050x; 1.3050x over previous
"""Trainium2 Bass kernel for CounterfactualMultiheadAttention (uniform type).

Math note: the reference computes w = softmax(masked scores), then replaces
every nonzero weight with row_sum(w)/nnz(w).  Softmax rows sum to 1 and are
nonzero exactly at unmasked key positions (score spread << 87, so exp never
underflows to 0), hence

    attn_weights[b,h,q,k] = (1 - mask[b,k]) / cnt[b]        (indep. of h, q)
    ctx[b,h,q,:]          = masked-mean over k of V[b,k,:]   (indep. of q)
    output[b,q,:] = (masked_mean(value[b]) @ Wv.T + bv) @ Wo.T + bo

so query/key/Wq/bq/Wk/bk cancel out entirely.  The kernel computes the
masked mean + two matvecs on-device and materializes the broadcast outputs.
The kernel is DMA-byte-bound; every design choice minimizes HBM traffic.

Sharding over 8 cores: core c handles batch b=c//2 and half-index hg=c%2;
it writes attn_weights for heads [8*hg, 8*hg+8) of batch b and the output
column block output[b, :, 512*hg : 512*hg+512].  Each core receives only its
half of Wo/bo (per-core input data), halving the Wo read.
"""

import sys

for _p in ("/opt/trn_rl_repo",):
    if _p not in sys.path:
        sys.path.insert(0, _p)

from contextlib import ExitStack

import numpy as np

import concourse.bass as bass
import concourse.mybir as mybir
import concourse.tile as tile
from concourse import bacc
from concourse.bass_utils import run_bass_kernel_spmd
from concourse.masks import make_identity

B, L, D = 4, 1024, 1024
H, HPC = 16, 8  # total heads, heads per core
P = 128
NCH = D // P  # 8 chunks of 128
OHALF = D // 2  # output features per core (column split)
NWO = OHALF // P  # 4
F32 = mybir.dt.float32
U8 = mybir.dt.uint8
ALU = mybir.AluOpType


def build_program():
    nc = bacc.Bacc("TRN2", target_bir_lowering=False, debug=False, num_devices=8)

    value_b = nc.dram_tensor("value_b", [L, D], F32, kind="ExternalInput")
    mask_b = nc.dram_tensor("mask_b", [L], U8, kind="ExternalInput")
    wv = nc.dram_tensor("wv", [D, D], F32, kind="ExternalInput")
    bv = nc.dram_tensor("bv", [D], F32, kind="ExternalInput")
    wo_h = nc.dram_tensor("wo_h", [OHALF, D], F32, kind="ExternalInput")
    bo_h = nc.dram_tensor("bo_h", [OHALF], F32, kind="ExternalInput")
    attn_out = nc.dram_tensor("attn_out", [HPC, L, L], F32, kind="ExternalOutput")
    out_cols = nc.dram_tensor("out_cols", [L, OHALF], F32, kind="ExternalOutput")

    with tile.TileContext(nc) as tc, ExitStack() as ctx:
        singles = ctx.enter_context(tc.tile_pool(name="singles", bufs=1))
        psum = ctx.enter_context(tc.tile_pool(name="psum", bufs=1, space="PSUM"))

        # ---- attn row = (1 - mask) / cnt, replicated on all 128 partitions ----
        mask_row = singles.tile([P, L], U8)
        nc.scalar.dma_start(out=mask_row[:], in_=bass.AP(tensor=mask_b, offset=0, ap=[[0, P], [1, L]]))
        valid_row = singles.tile([P, L], F32)
        nc.vector.tensor_scalar(
            out=valid_row[:], in0=mask_row[:], scalar1=0, scalar2=None,
            op0=ALU.is_equal,
        )
        cnt = singles.tile([P, 1], F32)
        nc.vector.reduce_sum(out=cnt[:], in_=valid_row[:], axis=mybir.AxisListType.X)
        inv = singles.tile([P, 1], F32)
        nc.vector.reciprocal(out=inv[:], in_=cnt[:])
        rowtile = singles.tile([P, L], F32)
        nc.vector.tensor_scalar_mul(rowtile[:], valid_row[:], inv[:])

        # ---- attn_weights: 8 head-slabs x 4 MiB broadcast-writes.
        # Source AP replicates the 512 KiB row tile 8x via a step-0 dim.
        attn_view = attn_out.ap().rearrange("h (n p) d -> h p n d", p=P)
        rt = rowtile[:]
        rt_rep8 = bass.AP(tensor=rt.tensor, offset=rt.offset,
                          ap=[rt.ap[0], [0, NCH], rt.ap[1]])
        for h in range(HPC):
            nc.sync.dma_start(out=attn_view[h], in_=rt_rep8)

        # ---- mask column layout + weights for the masked mean ----
        wcol_u8 = singles.tile([P, NCH], U8)
        nc.scalar.dma_start(out=wcol_u8[:], in_=mask_b.ap().rearrange("(n p) -> p n", p=P))
        wcol = singles.tile([P, NCH], F32)
        nc.vector.tensor_scalar(
            out=wcol[:], in0=wcol_u8[:], scalar1=0, scalar2=None, op0=ALU.is_equal,
        )
        wcoln = singles.tile([P, NCH], F32)
        nc.vector.tensor_scalar_mul(wcoln[:], wcol[:], inv[:])

        ones = singles.tile([1, P], F32)
        nc.vector.memset(ones[:], 1.0)
        identity = singles.tile([P, P], F32)
        make_identity(nc, identity[:])

        # ---- mv = masked mean of value rows (PE contraction over k) ----
        value_sb = singles.tile([P, NCH, D], F32)
        nc.scalar.dma_start(out=value_sb[:], in_=value_b.ap().rearrange("(n p) d -> p n d", p=P))
        mv_psum = psum.tile([1, L], F32, tag="rowbuf")
        for half in range(2):
            sl = slice(half * 512, (half + 1) * 512)
            for n in range(NCH):
                nc.tensor.matmul(
                    mv_psum[:, sl], lhsT=wcoln[:, n : n + 1], rhs=value_sb[:, n, sl],
                    start=(n == 0), stop=(n == NCH - 1),
                )
        mv_row = singles.tile([1, L], F32)
        nc.scalar.copy(out=mv_row[:], in_=mv_psum[:])
        # broadcast row to all partitions via PE outer product (K=1)
        mv_bc_psum = psum.tile([P, L], F32, tag="bcastbuf")
        for half in range(2):
            sl = slice(half * 512, (half + 1) * 512)
            nc.tensor.matmul(mv_bc_psum[:, sl], lhsT=ones[:], rhs=mv_row[:, sl])
        mv_bcast = singles.tile([P, L], F32)
        nc.vector.tensor_copy(mv_bcast[:], mv_bc_psum[:])

        # ---- vbar = mv @ Wv.T + bv  (DVE dot-rows, streamed wv chunks) ----
        bv_col = singles.tile([P, NCH], F32)
        nc.scalar.dma_start(out=bv_col[:], in_=bv.ap().rearrange("(n p) -> p n", p=P))
        wv_tile = singles.tile([P, NCH, D], F32)
        nc.scalar.dma_start(out=wv_tile[:], in_=wv.ap().rearrange("(n p) d -> p n d", p=P))
        vbar_acc = singles.tile([P, NCH], F32)
        scratch = singles.tile([P, D], F32)
        for n in range(NCH):
            nc.vector.tensor_mul(scratch[:], wv_tile[:, n, :], mv_bcast[:])
            nc.vector.reduce_sum(out=vbar_acc[:, n : n + 1], in_=scratch[:],
                                 axis=mybir.AxisListType.X)
        vbar_col = singles.tile([P, NCH], F32)
        nc.vector.tensor_add(vbar_col[:], vbar_acc[:], bv_col[:])

        # column [128, 8] -> row [1, 1024] via PE transpose, then PE broadcast
        vbar_rpsum = psum.tile([1, NCH, P], F32, tag="rowbuf")
        for n in range(NCH):
            nc.tensor.transpose(vbar_rpsum[:, n, :], vbar_col[:, n : n + 1], identity[:])
        vbar_row = singles.tile([1, D], F32)
        nc.scalar.copy(out=vbar_row[:], in_=vbar_rpsum[:].rearrange("o n p -> o (n p)"))
        vbar_bc_psum = psum.tile([P, D], F32, tag="bcastbuf")
        for half in range(2):
            sl = slice(half * 512, (half + 1) * 512)
            nc.tensor.matmul(vbar_bc_psum[:, sl], lhsT=ones[:], rhs=vbar_row[:, sl])
        vbar_bcast = singles.tile([P, D], F32)
        nc.vector.tensor_copy(vbar_bcast[:], vbar_bc_psum[:])

        # ---- orow_half = vbar @ Wo_half.T + bo_half  (this core's 512 cols) ----
        bo_col = singles.tile([P, NWO], F32)
        nc.scalar.dma_start(out=bo_col[:], in_=bo_h.ap().rearrange("(m p) -> p m", p=P))
        wo_tile = singles.tile([P, NWO, D], F32)
        nc.scalar.dma_start(out=wo_tile[:], in_=wo_h.ap().rearrange("(m p) j -> p m j", p=P))
        orow_acc = singles.tile([P, NWO], F32)
        scratch2 = singles.tile([P, D], F32)
        for m in range(NWO):
            nc.vector.tensor_mul(scratch2[:], wo_tile[:, m, :], vbar_bcast[:])
            nc.vector.reduce_sum(out=orow_acc[:, m : m + 1], in_=scratch2[:],
                                 axis=mybir.AxisListType.X)
        orow_col = singles.tile([P, NWO], F32)
        nc.vector.tensor_add(orow_col[:], orow_acc[:], bo_col[:])

        orow_rpsum = psum.tile([1, NWO, P], F32, tag="rowbuf")
        for m in range(NWO):
            nc.tensor.transpose(orow_rpsum[:, m, :], orow_col[:, m : m + 1], identity[:])
        orow_row = singles.tile([1, OHALF], F32)
        nc.scalar.copy(out=orow_row[:], in_=orow_rpsum[:].rearrange("o n p -> o (n p)"))
        orow_bc_psum = psum.tile([P, OHALF], F32, tag="bcastbuf")
        nc.tensor.matmul(orow_bc_psum[:], lhsT=ones[:], rhs=orow_row[:])
        orow_bcast = singles.tile([P, OHALF], F32)
        nc.vector.tensor_copy(orow_bcast[:], orow_bc_psum[:])

        # ---- output column block: [1024, 512] broadcast rows, one DMA ----
        out_view = out_cols.ap().rearrange("(n p) i -> p n i", p=P)
        ob = orow_bcast[:]
        ob_rep8 = bass.AP(tensor=ob.tensor, offset=ob.offset,
                          ap=[ob.ap[0], [0, L // P], ob.ap[1]])
        nc.sync.dma_start(out=out_view, in_=ob_rep8)

    nc.compile()
    return nc


_NC = None


def _get_nc():
    global _NC
    if _NC is None:
        _NC = build_program()
    return _NC


_LAST_RESULTS = None  # BassKernelResults of the most recent run (for profiling)


def kernel(query, key, value, key_padding_mask, Wq, bq, Wk, bk, Wv, bv, Wo, bo,
           _trace=False):
    nc = _get_nc()
    Wv = np.ascontiguousarray(np.asarray(Wv, dtype=np.float32))
    bvv = np.ascontiguousarray(np.asarray(bv, dtype=np.float32))
    Wo = np.asarray(Wo, dtype=np.float32)
    boo = np.asarray(bo, dtype=np.float32)
    value = np.asarray(value, dtype=np.float32)
    masku8 = np.asarray(key_padding_mask).astype(np.uint8)

    in_maps = []
    for c in range(8):
        b, hg = c // 2, c % 2
        in_maps.append({
            "value_b": np.ascontiguousarray(value[b]),
            "mask_b": np.ascontiguousarray(masku8[b]),
            "wv": Wv, "bv": bvv,
            "wo_h": np.ascontiguousarray(Wo[hg * OHALF : (hg + 1) * OHALF]),
            "bo_h": np.ascontiguousarray(boo[hg * OHALF : (hg + 1) * OHALF]),
        })

    global _LAST_RESULTS
    _LAST_RESULTS = run_bass_kernel_spmd(
        nc, in_maps, core_ids=list(range(8)), trace=_trace,
    )
    res = _LAST_RESULTS.results

    output = np.empty((B, L, D), dtype=np.float32)
    attn = np.empty((B, H, L, L), dtype=np.float32)
    for c in range(8):
        b, hg = c // 2, c % 2
        attn[b, hg * HPC : (hg + 1) * HPC] = res[c]["attn_out"]
        output[b, :, hg * OHALF : (hg + 1) * OHALF] = res[c]["out_cols"]
    return output, attn


# revision 12
# speedup vs baseline: 3.1511x; 2.4146x over previous
"""Trainium2 Bass kernel for CounterfactualMultiheadAttention (uniform type).

Math note: the reference computes w = softmax(masked scores), then replaces
every nonzero weight with row_sum(w)/nnz(w).  Softmax rows sum to 1 and are
nonzero exactly at unmasked key positions (score spread << 87, so exp never
underflows to 0), hence

    attn_weights[b,h,q,k] = (1 - mask[b,k]) / cnt[b]        (indep. of h, q)
    ctx[b,h,q,:]          = masked-mean over k of V[b,k,:]   (indep. of q)
    output[b,q,:] = (masked_mean(value[b]) @ Wv.T + bv) @ Wo.T + bo

so query/key/Wq/bq/Wk/bk cancel out entirely.  The kernel computes the
masked mean + two matvecs on-device and materializes the broadcast outputs.
The kernel is DMA-byte-bound; every design choice minimizes HBM traffic.

Sharding over 8 cores: core c handles batch b=c//2 and half-index hg=c%2;
it writes attn_weights for heads [8*hg, 8*hg+8) of batch b and the output
column block output[b, :, 512*hg : 512*hg+512].  Each core receives only its
half of Wo/bo (per-core input data), halving the Wo read.
"""

import sys

for _p in ("/opt/trn_rl_repo",):
    if _p not in sys.path:
        sys.path.insert(0, _p)

from contextlib import ExitStack

import numpy as np

import concourse.bass as bass
import concourse.mybir as mybir
import concourse.tile as tile
from concourse import bacc
from concourse.bass_utils import run_bass_kernel_spmd
from concourse.masks import make_identity

B, L, D = 4, 1024, 1024
H, HPC = 16, 8  # total heads, heads per core
P = 128
NCH = D // P  # 8 chunks of 128
OHALF = D // 2  # output features per core (column split)
NWO = OHALF // P  # 4
F32 = mybir.dt.float32
U8 = mybir.dt.uint8
ALU = mybir.AluOpType


def build_program():
    nc = bacc.Bacc("TRN2", target_bir_lowering=False, debug=False, num_devices=8)

    value_b = nc.dram_tensor("value_b", [L, D], F32, kind="ExternalInput")
    mask_b = nc.dram_tensor("mask_b", [L], U8, kind="ExternalInput")
    wv = nc.dram_tensor("wv", [D, D], F32, kind="ExternalInput")
    bv = nc.dram_tensor("bv", [D], F32, kind="ExternalInput")
    wo_h = nc.dram_tensor("wo_h", [OHALF, D], F32, kind="ExternalInput")
    bo_h = nc.dram_tensor("bo_h", [OHALF], F32, kind="ExternalInput")
    attn_out = nc.dram_tensor("attn_out", [HPC, L, L], F32, kind="ExternalOutput")
    out_cols = nc.dram_tensor("out_cols", [L, OHALF], F32, kind="ExternalOutput")

    with tile.TileContext(nc) as tc, ExitStack() as ctx:
        singles = ctx.enter_context(tc.tile_pool(name="singles", bufs=1))
        psum = ctx.enter_context(tc.tile_pool(name="psum", bufs=1, space="PSUM"))

        # ---- attn row = (1 - mask) / cnt, replicated on all 128 partitions ----
        mask_row = singles.tile([P, L], U8)
        nc.scalar.dma_start(out=mask_row[:], in_=bass.AP(tensor=mask_b, offset=0, ap=[[0, P], [1, L]]))
        valid_row = singles.tile([P, L], F32)
        cnt = singles.tile([P, 1], F32)
        # fused: valid = (mask == 0), cnt = row-sum(valid) in one DVE op
        nc.vector.tensor_scalar(
            out=valid_row[:], in0=mask_row[:], scalar1=0, scalar2=0.0,
            op0=ALU.is_equal, op1=ALU.add, accum_out=cnt[:],
        )
        inv = singles.tile([P, 1], F32)
        nc.vector.reciprocal(out=inv[:], in_=cnt[:])
        rowtile = singles.tile([P, L], F32)
        nc.vector.tensor_scalar_mul(rowtile[:], valid_row[:], inv[:])

        # ---- attn_weights: 8 head-slabs x 4 MiB broadcast-writes.
        # Source AP replicates the 512 KiB row tile 8x via a step-0 dim.
        attn_view = attn_out.ap().rearrange("h (n p) d -> h p n d", p=P)
        rt = rowtile[:]
        rt_rep8 = bass.AP(tensor=rt.tensor, offset=rt.offset,
                          ap=[rt.ap[0], [0, NCH], rt.ap[1]])
        for h in range(HPC):
            nc.sync.dma_start(out=attn_view[h], in_=rt_rep8)

        # ---- mask column layout + weights for the masked mean ----
        wcol_u8 = singles.tile([P, NCH], U8)
        nc.scalar.dma_start(out=wcol_u8[:], in_=mask_b.ap().rearrange("(n p) -> p n", p=P))
        wcol = singles.tile([P, NCH], F32)
        nc.vector.tensor_scalar(
            out=wcol[:], in0=wcol_u8[:], scalar1=0, scalar2=None, op0=ALU.is_equal,
        )
        wcoln = singles.tile([P, NCH], F32)
        nc.vector.tensor_scalar_mul(wcoln[:], wcol[:], inv[:])

        ones = singles.tile([1, P], F32)
        nc.vector.memset(ones[:], 1.0)
        identity = singles.tile([P, P], F32)
        make_identity(nc, identity[:])

        # ---- mv = masked mean of value rows (PE contraction over k) ----
        value_sb = singles.tile([P, NCH, D], F32)
        nc.scalar.dma_start(out=value_sb[:], in_=value_b.ap().rearrange("(n p) d -> p n d", p=P))
        mv_psum = psum.tile([1, L], F32, tag="rowbuf")
        for half in range(2):
            sl = slice(half * 512, (half + 1) * 512)
            for n in range(NCH):
                nc.tensor.matmul(
                    mv_psum[:, sl], lhsT=wcoln[:, n : n + 1], rhs=value_sb[:, n, sl],
                    start=(n == 0), stop=(n == NCH - 1),
                )
        mv_row = singles.tile([1, L], F32)
        nc.scalar.copy(out=mv_row[:], in_=mv_psum[:])
        # broadcast row to all partitions via PE outer product (K=1)
        mv_bc_psum = psum.tile([P, L], F32, tag="bcastbuf")
        for half in range(2):
            sl = slice(half * 512, (half + 1) * 512)
            nc.tensor.matmul(mv_bc_psum[:, sl], lhsT=ones[:], rhs=mv_row[:, sl])
        mv_bcast = singles.tile([P, L], F32)
        nc.vector.tensor_copy(mv_bcast[:], mv_bc_psum[:])

        # ---- vbar = mv @ Wv.T + bv  (DVE dot-rows, streamed wv chunks) ----
        bv_col = singles.tile([P, NCH], F32)
        nc.scalar.dma_start(out=bv_col[:], in_=bv.ap().rearrange("(n p) -> p n", p=P))
        wv_tile = singles.tile([P, NCH, D], F32)
        nc.scalar.dma_start(out=wv_tile[:], in_=wv.ap().rearrange("(n p) d -> p n d", p=P))
        vbar_acc = singles.tile([P, NCH], F32)
        scratch = singles.tile([P, D], F32)
        for n in range(NCH):
            nc.vector.tensor_mul(scratch[:], wv_tile[:, n, :], mv_bcast[:])
            nc.vector.reduce_sum(out=vbar_acc[:, n : n + 1], in_=scratch[:],
                                 axis=mybir.AxisListType.X)
        vbar_col = singles.tile([P, NCH], F32)
        nc.vector.tensor_add(vbar_col[:], vbar_acc[:], bv_col[:])

        # column [128, 8] -> row [1, 1024] via PE transpose, then PE broadcast
        vbar_rpsum = psum.tile([1, NCH, P], F32, tag="rowbuf")
        for n in range(NCH):
            nc.tensor.transpose(vbar_rpsum[:, n, :], vbar_col[:, n : n + 1], identity[:])
        vbar_row = singles.tile([1, D], F32)
        nc.scalar.copy(out=vbar_row[:], in_=vbar_rpsum[:].rearrange("o n p -> o (n p)"))
        vbar_bc_psum = psum.tile([P, D], F32, tag="bcastbuf")
        for half in range(2):
            sl = slice(half * 512, (half + 1) * 512)
            nc.tensor.matmul(vbar_bc_psum[:, sl], lhsT=ones[:], rhs=vbar_row[:, sl])
        vbar_bcast = singles.tile([P, D], F32)
        nc.vector.tensor_copy(vbar_bcast[:], vbar_bc_psum[:])

        # ---- orow_half = vbar @ Wo_half.T + bo_half  (this core's 512 cols) ----
        bo_col = singles.tile([P, NWO], F32)
        nc.scalar.dma_start(out=bo_col[:], in_=bo_h.ap().rearrange("(m p) -> p m", p=P))
        wo_tile = singles.tile([P, NWO, D], F32)
        nc.scalar.dma_start(out=wo_tile[:], in_=wo_h.ap().rearrange("(m p) j -> p m j", p=P))
        orow_acc = singles.tile([P, NWO], F32)
        scratch2 = singles.tile([P, D], F32)
        for m in range(NWO):
            nc.vector.tensor_mul(scratch2[:], wo_tile[:, m, :], vbar_bcast[:])
            nc.vector.reduce_sum(out=orow_acc[:, m : m + 1], in_=scratch2[:],
                                 axis=mybir.AxisListType.X)
        orow_col = singles.tile([P, NWO], F32)
        nc.vector.tensor_add(orow_col[:], orow_acc[:], bo_col[:])

        orow_rpsum = psum.tile([1, NWO, P], F32, tag="rowbuf")
        for m in range(NWO):
            nc.tensor.transpose(orow_rpsum[:, m, :], orow_col[:, m : m + 1], identity[:])
        orow_row = singles.tile([1, OHALF], F32)
        nc.scalar.copy(out=orow_row[:], in_=orow_rpsum[:].rearrange("o n p -> o (n p)"))
        orow_bc_psum = psum.tile([P, OHALF], F32, tag="bcastbuf")
        nc.tensor.matmul(orow_bc_psum[:], lhsT=ones[:], rhs=orow_row[:])
        orow_bcast = singles.tile([P, OHALF], F32)
        nc.vector.tensor_copy(orow_bcast[:], orow_bc_psum[:])

        # ---- output column block: [1024, 512] broadcast rows, one DMA ----
        out_view = out_cols.ap().rearrange("(n p) i -> p n i", p=P)
        ob = orow_bcast[:]
        ob_rep8 = bass.AP(tensor=ob.tensor, offset=ob.offset,
                          ap=[ob.ap[0], [0, L // P], ob.ap[1]])
        # scalar ring: doesn't queue behind the 8 attn slab DMAs on sync
        nc.scalar.dma_start(out=out_view, in_=ob_rep8)

    nc.compile()
    return nc


_NC = None


def _get_nc():
    global _NC
    if _NC is None:
        _NC = build_program()
    return _NC


_LAST_RESULTS = None  # BassKernelResults of the most recent run (for profiling)


def kernel(query, key, value, key_padding_mask, Wq, bq, Wk, bk, Wv, bv, Wo, bo,
           _trace=False):
    nc = _get_nc()
    Wv = np.ascontiguousarray(np.asarray(Wv, dtype=np.float32))
    bvv = np.ascontiguousarray(np.asarray(bv, dtype=np.float32))
    Wo = np.asarray(Wo, dtype=np.float32)
    boo = np.asarray(bo, dtype=np.float32)
    value = np.asarray(value, dtype=np.float32)
    masku8 = np.asarray(key_padding_mask).astype(np.uint8)

    in_maps = []
    for c in range(8):
        b, hg = c // 2, c % 2
        in_maps.append({
            "value_b": np.ascontiguousarray(value[b]),
            "mask_b": np.ascontiguousarray(masku8[b]),
            "wv": Wv, "bv": bvv,
            "wo_h": np.ascontiguousarray(Wo[hg * OHALF : (hg + 1) * OHALF]),
            "bo_h": np.ascontiguousarray(boo[hg * OHALF : (hg + 1) * OHALF]),
        })

    global _LAST_RESULTS
    _LAST_RESULTS = run_bass_kernel_spmd(
        nc, in_maps, core_ids=list(range(8)), trace=_trace,
    )
    res = _LAST_RESULTS.results

    output = np.empty((B, L, D), dtype=np.float32)
    attn = np.empty((B, H, L, L), dtype=np.float32)
    for c in range(8):
        b, hg = c // 2, c % 2
        attn[b, hg * HPC : (hg + 1) * HPC] = res[c]["attn_out"]
        output[b, :, hg * OHALF : (hg + 1) * OHALF] = res[c]["out_cols"]
    return output, attn
